# revision 35
# baseline (speedup 1.0000x reference)
"""Trainium2 Bass kernel for additive (Bahdanau) attention.

  context[b] = sum_t softmax_t( v . tanh(We @ enc[b,t] + Wd @ dec[b] + bias) ) * enc[b,t]

Shapes (hardcoded): enc_out [64, 2048, 1024] f32, dec_state [64, 1024] f32,
W_weight [1024, 2048], W_bias [1024], v_weight [1, 1024].  Output [64, 1024].

Sharding: data-parallel over batch across 8 NeuronCores (8 batches/core).

Design v3 (measured-rate balance of all four engines; v1 in
kernel_v1_baseline.py; ~323us/core vs v1's ~335us in the same chip power
state, both at the 2.4GHz fast state):
- Host prep: enc cast twice -- bf16 [b, tl, i, e] (ctx moving stream) and
  fp8-e4m3 [b, el, i, j, tl] (proj stationary).  We^T scaled x32 in fp8
  [el, j, d] pairs (dodges e4m3 subnormals; tanh's scale=1/32 undoes it).
  z = 32*(Wd @ dec + bias) bf16 replicated to 128 partitions.
- proj: 8 fp8 DoubleRow matmuls per tile into ONE [128,1024] f32 PSUM tile
  spanning 2 adjacent banks (ring of 3).  Alternating banks per matmul
  removes the PSUM accumulate back-pressure: DR issue rate is the 216ns
  streaming floor (vs 259ns when chaining one bank).  The z-add is then a
  single DVE tensor_add over all 1024 cols.
- tanh on ACT over FOUR tiles at once ([128,4096] in-place on a quad
  e-buffer): 0.92us/tile vs 1.36 single (the ~0.5us/instr ACT overhead
  amortizes).  Last quad runs per-tile tanh instead to shorten the drain
  chain (ACT is idle there).
- v-dot by measured rates: DVE scalar_tensor_tensor w/accum fuses
  mult+reduce on [0:512] (~1.3ns/col); Pool tensor_mul takes [512:1024]
  (~2.1ns/col, otherwise-idle engine); ACT Copy+accum reduces that half.
  The quad's four STT/mults fire 2+2 over the two steps after its tanh
  (a single 4-burst head-of-line-blocks the DVE queue behind the 3.7us
  tanh).  s = s0+s1 on Pool (keeps the DVE queue clear), one ACT exp per
  4 tiles.  Last 8 tiles run the whole v-dot on DVE so the ACT/Pool
  queues drain early.  NOTE: DVE tensor_tensor_reduce would do the fused
  v-dot in one op but crashes the exec unit on this HW (NRT 101).
- ctx += p^T @ X: the two N=512 bf16 matmuls of a tile write partitions 0
  and 32 of the SAME PSUM bank = different PE column groups, so they run
  concurrently (~215ns/pair; tile_position auto-derives from the output
  base partition).  The batch's l-sum matmul rides partition 64.  ctx
  emits per 2 tiles; grouping 4+ serializes the PSUM accumulate chains.
- Softmax needs no max-subtraction (|scores| <= sum|v| <= 32).
- Emission-order invariant: an instruction reading p_all/s/s1 must be
  EMITTED after its producer (the Tile framework orders by program
  order); violating this reads stale SBUF and can hard-crash with the
  col-grouped/2-bank variants.
"""
import sys

sys.path.insert(0, "/opt/trn_rl_repo")

K_PJ2 = True     # proj into one [128,1024] 2-bank PSUM tile
K_CTXCG = True   # ctx halves col-grouped into one shared PSUM bank
K_TANHQ = True   # tanh 4 tiles per ACT instruction

from contextlib import ExitStack

import ml_dtypes
import numpy as np

import concourse.tile as tile
from concourse import bacc, mybir
from concourse.bass_utils import run_bass_kernel_spmd

F32 = mybir.dt.float32
BF16 = mybir.dt.bfloat16
FP8 = mybir.dt.float8e4
NP_FP8 = ml_dtypes.float8_e4m3
NP_BF16 = ml_dtypes.bfloat16
DR = mybir.MatmulPerfMode.DoubleRow

B, T, E, D = 64, 2048, 1024, 1024
CORES = 8
BL = B // CORES           # batches per core (8)
P = 128                   # partitions
TT = T // P               # t-tiles per batch (16)
ET = E // P               # e-blocks per row-tile (8)
QUAD = 4                  # t-tiles fetched per DMA instr / tanh'd per ACT instr
PREFETCH_QUADS = 4
WSCALE = 32.0             # fp8 subnormal-avoidance scale on We^T and z
ASPLIT = 512              # v-dot cols fused on DVE; rest Pool-mult + ACT-reduce

# pipeline lags (in t-tiles) behind emit_proj(k)
L_ZADD = 1                # DVE z-add of tile k-1 (quad-slice write)
# quad-tanh of tiles [k-4..k-1] fires when (k-1) completes a quad
# quad-synchronized epilogue: when tile k-1 completes quad q, the whole
# quad's vdot fires at once, reduces next step, exp the step after.
L_VDOT = 2                # quad vdot burst fires one step after its tanh
L_RED = 3                 # quad reduce one step after quad vdot
L_EXP = 4                 # quad exp two steps after quad vdot
L_CTX = 7                 # ctx pair (k-7, k-6) on even kc
L_END = 9


def _build_kernel(bl=BL, t_tiles=TT):
    nc = bacc.Bacc(
        "TRN2",
        target_bir_lowering=False,
        debug=False,
        num_devices=CORES,
    )

    # [b, tl, i, e]: x16[b, tl, i, :] = enc[b, i*128+tl, :] in bf16
    x16 = nc.declare_dram_parameter("x16", [bl, P, t_tiles, E], BF16, isOutput=False)
    # [b, el, i, j, tl]: xt8[b, el, i, j, tl] = enc[b, i*128+tl, j*128+el] in fp8
    xt8 = nc.declare_dram_parameter("xt8", [bl, P, t_tiles, ET, P], FP8, isOutput=False)
    # [el, j, d]: wet8[el, j, d] = 32 * We[d, j*128+el] in fp8
    wet8 = nc.declare_dram_parameter("wet8", [P, ET, D], FP8, isOutput=False)
    # [k, b, d]: 32 * z[b, d] in bf16, replicated over k partitions
    zrep16 = nc.declare_dram_parameter("zrep16", [P, bl, D], BF16, isOutput=False)
    v16 = nc.declare_dram_parameter("v16", [P, D], BF16, isOutput=False)
    onesc = nc.declare_dram_parameter("onesc", [P, 1], BF16, isOutput=False)
    out = nc.declare_dram_parameter("ctx_out", [bl, E], F32, isOutput=True)

    n_quads_total = bl * t_tiles // QUAD

    with tile.TileContext(nc) as tc, ExitStack() as ctx:
        const = ctx.enter_context(tc.tile_pool(name="const", bufs=1))
        xq_pool = ctx.enter_context(tc.tile_pool(name="xq", bufs=6))
        xtq_pool = ctx.enter_context(tc.tile_pool(name="xtq", bufs=6))
        epool = ctx.enter_context(tc.tile_pool(name="e", bufs=5))
        small = ctx.enter_context(tc.tile_pool(name="small", bufs=2))

        n_proj_bufs = 3 if K_PJ2 else 5
        ps_proj = ctx.enter_context(
            tc.tile_pool(name="ps_proj", bufs=n_proj_bufs, space="PSUM"))
        ps_ctx = ctx.enter_context(tc.tile_pool(name="ps_ctx", bufs=2, space="PSUM"))
        if not (K_PJ2 and K_CTXCG):
            ps_misc = ctx.enter_context(
                tc.tile_pool(name="ps_misc", bufs=1, space="PSUM"))

        # ---- resident constants.  Ordered so proj(0)'s inputs land first.
        xq_tiles = {}
        xtq_tiles = {}

        def fetch_quad(q, skip_xq=False):
            b, qi = divmod(q, t_tiles // QUAD)
            if not skip_xq:
                xq = xq_pool.tile([P, QUAD, E], BF16, tag="xq")
                nc.sync.dma_start(xq[:], x16[b, :, QUAD * qi : QUAD * (qi + 1), :])
                xq_tiles[q] = xq
            xtq = xtq_pool.tile([P, QUAD, ET, P], FP8, tag="xtq")
            nc.sync.dma_start(xtq[:], xt8[b, :, QUAD * qi : QUAD * (qi + 1), :, :])
            xtq_tiles[q] = xtq

        fetch_quad(0, skip_xq=True)
        wet_t = []
        z_t = [const.tile([P, D], BF16, name=f"z{b}") for b in range(bl)]
        for pr in range(ET // 2):
            wt = const.tile([P, 2, D], FP8, name=f"wet{pr}")
            nc.sync.dma_start(wt[:], wet8[:, 2 * pr : 2 * pr + 2, :])
            wet_t.append(wt)
            if pr == 1:
                nc.sync.dma_start(z_t[0][:], zrep16[:, 0])
        v_sb = const.tile([P, D], BF16)
        nc.sync.dma_start(v_sb[:], v16[:])
        onesc_sb = const.tile([P, 1], BF16)
        nc.sync.dma_start(onesc_sb[:], onesc[:])
        # scratch sinks (contents never read)
        junk = const.tile([P, D], BF16, name="junk")
        prod = const.tile([P, 4, D - ASPLIT], BF16, name="prod")

        # ---- per-batch state ------------------------------------------------
        total = bl * t_tiles
        state = {}
        equads = {}   # quad index -> [128, QUAD, D] e-buffer

        def get_state(b):
            if b not in state:
                state[b] = dict(
                    s0=small.tile([P, t_tiles], F32, tag="s0", name=f"s0_{b}"),
                    s1=small.tile([P, t_tiles], F32, tag="s1", name=f"s1_{b}"),
                    s=small.tile([P, t_tiles], F32, tag="s", name=f"s_{b}"),
                    p_all=small.tile([P, t_tiles], BF16, tag="p", name=f"p_all_{b}"),
                    # one PSUM bank: ctx halves at partitions 0 / 32, l at 64
                    ctxb=(ps_ctx.tile([P, 512], F32, tag="ps_ctx", name=f"ctxb_{b}")
                          if K_CTXCG else None),
                    ctx0=(None if K_CTXCG else
                          ps_ctx.tile([1, 512], F32, tag="ps_ctx", name=f"ctx0_{b}")),
                    ctx1=(None if K_CTXCG else
                          ps_ctx.tile([1, 512], F32, tag="ps_ctx", name=f"ctx1_{b}")),
                    proj_ps=[None] * t_tiles,
                )
            return state[b]

        def emit_proj(b, i):
            # proj[t, d] = sum_e x[t, e] * 32*WeT[e, d], fp8 DoubleRow,
            # both 512-halves into one [128,1024] 2-bank PSUM tile
            st = get_state(b)
            k = b * t_tiles + i
            q, qi = divmod(k, QUAD)
            xtq = xtq_tiles[q]
            if K_PJ2:
                pj = ps_proj.tile([P, D], F32, tag="ps_proj", name=f"pj_{b}_{i}")
                pjs = [pj[:, 0:512], pj[:, 512:1024]]
                st["proj_ps"][i] = [pj]
            else:
                pj0 = ps_proj.tile([P, 512], F32, tag="ps_proj", name=f"pj0_{b}_{i}")
                pj1 = ps_proj.tile([P, 512], F32, tag="ps_proj", name=f"pj1_{b}_{i}")
                pjs = [pj0[:], pj1[:]]
                st["proj_ps"][i] = [pj0, pj1]
            for pr in range(ET // 2):
                lhs = xtq[:, qi, 2 * pr : 2 * pr + 2, :]
                for h in range(2):
                    nc.tensor.matmul(
                        pjs[h], lhs, wet_t[pr][:, :, 512 * h : 512 * (h + 1)],
                        start=(pr == 0), stop=(pr == ET // 2 - 1), perf_mode=DR,
                    )

        def emit_zadd(k):
            # e = (proj + 32z) bf16 into this quad's slice; one [128,1024]
            # DVE op spanning both PSUM banks
            b, i = divmod(k, t_tiles)
            st = get_state(b)
            q, qi = divmod(k, QUAD)
            if q not in equads:
                equads[q] = epool.tile([P, QUAD, D], BF16, tag="e", name=f"equad_{q}")
            pjl = st["proj_ps"][i]
            st["proj_ps"][i] = None
            if K_PJ2:
                nc.vector.tensor_add(equads[q][:, qi, :], pjl[0][:], z_t[b][:])
            else:
                for h in range(2):
                    sl = slice(512 * h, 512 * (h + 1))
                    nc.vector.tensor_add(
                        equads[q][:, qi, sl], pjl[h][:], z_t[b][:, sl]
                    )

        def emit_tanh_tile(k):
            # drain tail: per-tile tanh right after its z-add spreads the
            # last quad's work before proj ends (ACT is idle there)
            q, qi = divmod(k, QUAD)
            nc.scalar.activation(
                equads[q][:, qi, :], equads[q][:, qi, :],
                mybir.ActivationFunctionType.Tanh, scale=1.0 / WSCALE,
            )

        def emit_tanh_quad(q):
            # tanh over 4 tiles at once; ~0.98us/tile vs 1.36 single
            if K_TANHQ:
                nc.scalar.activation(
                    equads[q][:], equads[q][:], mybir.ActivationFunctionType.Tanh,
                    scale=1.0 / WSCALE,
                )
            else:
                for qi in range(QUAD):
                    nc.scalar.activation(
                        equads[q][:, qi, :], equads[q][:, qi, :],
                        mybir.ActivationFunctionType.Tanh, scale=1.0 / WSCALE,
                    )

        def emit_vdot(k):
            # cols [0:ASPLIT): DVE STT fused mult+accum -> s0
            # cols [ASPLIT:D): Pool mult -> prod (ACT reduces next step)
            # Drain tail (last 8 tiles): whole v-dot on DVE so the ACT/Pool
            # queues empty sooner -- the drain critical path is ACT.
            b, i = divmod(k, t_tiles)
            st = get_state(b)
            q, qi = divmod(k, QUAD)
            e_sb = equads[q]
            asp = D if k >= total - 8 else ASPLIT
            nc.vector.scalar_tensor_tensor(
                out=junk[:, 0:asp],
                in0=e_sb[:, qi, 0:asp],
                scalar=1.0,
                in1=v_sb[:, 0:asp],
                op0=mybir.AluOpType.mult,
                op1=mybir.AluOpType.mult,
                accum_out=st["s0"][:, i : i + 1],
            )
            if asp < D:
                nc.gpsimd.tensor_mul(
                    prod[:, k % 4, :], e_sb[:, qi, asp:D], v_sb[:, asp:D]
                )

        def emit_reduce(k):
            # ACT Copy+accum over Pool's product half -> s1
            if k >= total - 8:
                return
            b, i = divmod(k, t_tiles)
            st = get_state(b)
            nc.scalar.activation(
                prod[:, k % 4, :],
                prod[:, k % 4, :],
                mybir.ActivationFunctionType.Copy,
                accum_out=st["s1"][:, i : i + 1],
            )

        def emit_exp4(b, i0):
            # s = s0+s1 (DVE, [P,4]) then p = exp(s) (one ACT instr)
            st = get_state(b)
            sl = slice(i0, i0 + 4)
            if b * t_tiles + i0 >= total - 8:
                src_s = st["s0"]          # tail tiles: s0 holds the full dot
            else:
                nc.gpsimd.tensor_add(
                    st["s"][:, sl], st["s0"][:, sl], st["s1"][:, sl]
                )
                src_s = st["s"]
            nc.scalar.activation(
                st["p_all"][:, sl], src_s[:, sl],
                mybir.ActivationFunctionType.Exp,
            )

        def emit_ctx(b, i):
            # ctx_unnorm += p^T @ X; halves go to partitions 0 / 32 of one
            # PSUM bank = different PE column groups -> concurrent
            st = get_state(b)
            k = b * t_tiles + i
            q, qi = divmod(k, QUAD)
            xq = xq_tiles[q]
            p_col = st["p_all"][:, i : i + 1]
            for h in range(2):
                if K_CTXCG:
                    dst = st["ctxb"][32 * h : 32 * h + 1, :]
                else:
                    dst = st["ctx0" if h == 0 else "ctx1"][:]
                nc.tensor.matmul(
                    dst, p_col,
                    xq[:, qi, h * 512 : (h + 1) * 512],
                    start=(i == 0), stop=(i == t_tiles - 1),
                )

        def emit_batch_l(b):
            # stage 1: l = sum_t exp(s_t) via DVE reduce + 1-col matmul into
            # partition 64 of the ctx bank.  Emitted 3 steps before the
            # normalize so the cross-engine chain spreads across the queues
            # (the l matmul lands inside the same step's bf16 ctx group).
            st = get_state(b)
            l_part = small.tile([P, 1], BF16, tag="lp")
            with nc.allow_low_precision(reason="l partials; err ~0.2%/sqrt(128)"):
                nc.vector.tensor_reduce(
                    l_part[:], st["p_all"][:],
                    axis=mybir.AxisListType.X, op=mybir.AluOpType.add,
                )
            l_ps = st["ctxb"][64:65, 0:1]
            nc.tensor.matmul(l_ps, l_part[:], onesc_sb[:])

        def emit_batch_end(b):
            # stage 2: ctx = ctx_unnorm / l
            st = state.pop(b)
            l_ps = st["ctxb"][64:65, 0:1]
            c0, c1 = st["ctxb"][0:1, :], st["ctxb"][32:33, :]
            linv = small.tile([1, 1], F32, tag="linv")
            nc.vector.reciprocal(linv[:], l_ps)
            ctx_row = small.tile([1, E], F32, tag="ctxrow")
            nc.vector.tensor_scalar(
                ctx_row[:, 0:512], c0, linv[:], None, mybir.AluOpType.mult,
            )
            nc.scalar.activation(
                ctx_row[:, 512:E], c1,
                mybir.ActivationFunctionType.Copy, scale=linv[:],
            )
            nc.sync.dma_start(out[b : b + 1, :], ctx_row[:])

        # ---- main software pipeline over all (batch, t-tile) ----------------
        for k in range(total + L_END + 2):
            if k < total:
                emit_proj(*divmod(k, t_tiles))
            if k == 0:
                for q in range(1, PREFETCH_QUADS):
                    fetch_quad(q)
                # xq0 fetched late: first needed by ctx at step L_CTX, and
                # fetching it earlier delays the xtq quads proj stalls on
                xq0 = xq_pool.tile([P, QUAD, E], BF16, tag="xq")
                nc.sync.dma_start(xq0[:], x16[0, :, 0:QUAD, :])
                xq_tiles[0] = xq0
            if k % QUAD == 0:
                qf = k // QUAD + PREFETCH_QUADS
                if qf < n_quads_total:
                    fetch_quad(qf)
            if k % t_tiles == 8 and k // t_tiles + 1 < bl:
                b_next = k // t_tiles + 1
                nc.sync.dma_start(z_t[b_next][:], zrep16[:, b_next])
            kz = k - L_ZADD
            if 0 <= kz < total:
                emit_zadd(kz)
                if kz >= total - QUAD:
                    emit_tanh_tile(kz)
                elif kz % QUAD == QUAD - 1:
                    emit_tanh_quad(kz // QUAD)
            kv = k - L_VDOT
            if 0 <= kv < total:
                if kv >= total - QUAD:
                    if kv == total - QUAD:
                        emit_vdot(kv - 2)
                        emit_vdot(kv - 1)
                    emit_vdot(kv)
                elif kv % QUAD == QUAD - 1:
                    emit_vdot(kv - 3)
                    emit_vdot(kv - 2)
                elif kv % QUAD == 0 and kv > 0:
                    emit_vdot(kv - 2)
                    emit_vdot(kv - 1)
            kr = k - L_RED
            if 0 <= kr < total and kr % QUAD == QUAD - 1:
                for kk in range(kr - 3, kr + 1):
                    emit_reduce(kk)
            ke = k - L_EXP
            if 0 <= ke < total and ke % 4 == 3:
                b_e, i_e = divmod(ke, t_tiles)
                emit_exp4(b_e, (i_e // 4) * 4)
            kc = k - L_CTX
            if kc >= 0 and kc % 2 == 0:
                for kk in (kc, kc + 1):
                    if 0 <= kk < total:
                        emit_ctx(*divmod(kk, t_tiles))
            kb1 = k - L_END + 3
            if 0 <= kb1 < total and kb1 % t_tiles == t_tiles - 1:
                emit_batch_l(kb1 // t_tiles)
            kb = k - L_END
            if 0 <= kb < total and kb % t_tiles == t_tiles - 1:
                emit_batch_end(kb // t_tiles)
            # free the consumed e-quad once its last tile's reduce is done
            kq = k - L_RED - 1
            if kq >= 0 and kq % QUAD == QUAD - 1 and (kq // QUAD) in equads:
                equads.pop(kq // QUAD)

    nc.compile()
    return nc


def _prep_inputs(enc_out, dec_state, W_weight, W_bias, v_weight, bl=BL):
    """Host-side layout/dtype prep + per-core slicing."""
    enc_out = np.ascontiguousarray(enc_out, dtype=np.float32)
    dec_state = np.ascontiguousarray(dec_state, dtype=np.float32)
    W = np.asarray(W_weight, dtype=np.float32)

    # x16: [B, tl, i, e] bf16
    x16_h = np.ascontiguousarray(
        enc_out.reshape(B, TT, P, E).transpose(0, 2, 1, 3).astype(NP_BF16)
    )
    # xt8: [B, el, i, j, tl] fp8
    enc8 = enc_out.astype(NP_FP8)
    xt8_h = np.ascontiguousarray(
        enc8.reshape(B, TT, P, ET, P).transpose(0, 4, 1, 3, 2)
    )
    # wet8: [el, j, d], scaled by WSCALE to avoid e4m3 subnormals
    wet8_h = np.ascontiguousarray(
        (WSCALE * W[:, :E].T).astype(NP_FP8).reshape(ET, P, D).transpose(1, 0, 2)
    )
    # z = Wd @ dec + bias, scaled by WSCALE, bf16, replicated over k
    z_all = dec_state @ W[:, E:].T + np.asarray(W_bias, dtype=np.float32)  # [B, D]
    z16 = (WSCALE * z_all).astype(NP_BF16)
    v16_h = np.ascontiguousarray(
        np.broadcast_to(np.asarray(v_weight).astype(NP_BF16).reshape(1, D), (P, D))
    )
    onesc_h = np.ones((P, 1), dtype=NP_BF16)

    in_maps = []
    for c in range(CORES):
        sl = slice(c * bl, (c + 1) * bl)
        zrep_h = np.ascontiguousarray(np.broadcast_to(z16[None, sl], (P, bl, D)))
        in_maps.append(
            {
                "x16": x16_h[sl],
                "xt8": xt8_h[sl],
                "wet8": wet8_h,
                "zrep16": zrep_h,
                "v16": v16_h,
                "onesc": onesc_h,
            }
        )
    return in_maps


_NC_CACHE = {}


def _get_nc():
    if "nc" not in _NC_CACHE:
        _NC_CACHE["nc"] = _build_kernel()
    return _NC_CACHE["nc"]


def _run(inputs, trace=False, tmpdir=None):
    nc = _get_nc()
    in_maps = _prep_inputs(
        inputs["enc_out"],
        inputs["dec_state"],
        inputs["W_weight"],
        inputs["W_bias"],
        inputs["v_weight"],
    )
    res = run_bass_kernel_spmd(
        nc, in_maps, list(range(CORES)), trace=trace, tmpdir=tmpdir
    )
    out = np.concatenate(
        [np.asarray(res.results[c]["ctx_out"]) for c in range(CORES)], axis=0
    )
    return out.astype(np.float32, copy=False), res


def kernel(**inputs):
    out, _ = _run(inputs, trace=False)
    return out


if __name__ == "__main__":
    pass


# revision 36
# speedup vs baseline: 1.0387x; 1.0387x over previous
"""Trainium2 Bass kernel for additive (Bahdanau) attention.

  context[b] = sum_t softmax_t( v . tanh(We @ enc[b,t] + Wd @ dec[b] + bias) ) * enc[b,t]

Shapes (hardcoded): enc_out [64, 2048, 1024] f32, dec_state [64, 1024] f32,
W_weight [1024, 2048], W_bias [1024], v_weight [1, 1024].  Output [64, 1024].

Sharding: data-parallel over batch across 8 NeuronCores (8 batches/core).

Design v3 (measured-rate balance of all four engines; v1 in
kernel_v1_baseline.py; ~323us/core vs v1's ~335us in the same chip power
state, both at the 2.4GHz fast state):
- Host prep: enc cast twice -- bf16 [b, tl, i, e] (ctx moving stream) and
  fp8-e4m3 [b, el, i, j, tl] (proj stationary).  We^T scaled x32 in fp8
  [el, j, d] pairs (dodges e4m3 subnormals; tanh's scale=1/32 undoes it).
  z = 32*(Wd @ dec + bias) bf16 replicated to 128 partitions.
- proj: 8 fp8 DoubleRow matmuls per tile into ONE [128,1024] f32 PSUM tile
  spanning 2 adjacent banks (ring of 3).  Alternating banks per matmul
  removes the PSUM accumulate back-pressure: DR issue rate is the 216ns
  streaming floor (vs 259ns when chaining one bank).  The z-add is then a
  single DVE tensor_add over all 1024 cols.
- tanh on ACT over FOUR tiles at once ([128,4096] in-place on a quad
  e-buffer): 0.92us/tile vs 1.36 single (the ~0.5us/instr ACT overhead
  amortizes).  Last quad runs per-tile tanh instead to shorten the drain
  chain (ACT is idle there).
- v-dot by measured rates: DVE scalar_tensor_tensor w/accum fuses
  mult+reduce on [0:512] (~1.3ns/col); Pool tensor_mul takes [512:1024]
  (~2.1ns/col, otherwise-idle engine); ACT Copy+accum reduces that half.
  The quad's four STT/mults fire 2+2 over the two steps after its tanh
  (a single 4-burst head-of-line-blocks the DVE queue behind the 3.7us
  tanh).  s = s0+s1 on Pool (keeps the DVE queue clear), one ACT exp per
  4 tiles.  Last 8 tiles run the whole v-dot on DVE so the ACT/Pool
  queues drain early.  NOTE: DVE tensor_tensor_reduce would do the fused
  v-dot in one op but crashes the exec unit on this HW (NRT 101).
- ctx += p^T @ X: the two N=512 bf16 matmuls of a tile write partitions 0
  and 32 of the SAME PSUM bank = different PE column groups, so they run
  concurrently (~215ns/pair; tile_position auto-derives from the output
  base partition).  The batch's l-sum matmul rides partition 64.  ctx
  emits per 2 tiles; grouping 4+ serializes the PSUM accumulate chains.
- Softmax needs no max-subtraction (|scores| <= sum|v| <= 32).
- Emission-order invariant: an instruction reading p_all/s/s1 must be
  EMITTED after its producer (the Tile framework orders by program
  order); violating this reads stale SBUF and can hard-crash with the
  col-grouped/2-bank variants.
"""
import sys

sys.path.insert(0, "/opt/trn_rl_repo")

K_PJ2 = True     # proj into one [128,1024] 2-bank PSUM tile
K_CTXCG = True   # ctx halves col-grouped into one shared PSUM bank
K_TANHQ = True   # tanh 4 tiles per ACT instruction

from contextlib import ExitStack

import ml_dtypes
import numpy as np

import concourse.tile as tile
from concourse import bacc, mybir
from concourse.bass_utils import run_bass_kernel_spmd

F32 = mybir.dt.float32
BF16 = mybir.dt.bfloat16
FP8 = mybir.dt.float8e4
NP_FP8 = ml_dtypes.float8_e4m3
NP_BF16 = ml_dtypes.bfloat16
DR = mybir.MatmulPerfMode.DoubleRow

B, T, E, D = 64, 2048, 1024, 1024
CORES = 8
BL = B // CORES           # batches per core (8)
P = 128                   # partitions
TT = T // P               # t-tiles per batch (16)
ET = E // P               # e-blocks per row-tile (8)
QUAD = 4                  # t-tiles fetched per DMA instr / tanh'd per ACT instr
PREFETCH_QUADS = 4
WSCALE = 32.0             # fp8 subnormal-avoidance scale on We^T and z
ASPLIT = 512              # v-dot cols fused on DVE; rest Pool-mult + ACT-reduce

# pipeline lags (in t-tiles) behind emit_proj(k)
L_ZADD = 1                # DVE z-add of tile k-1 (quad-slice write)
# quad-tanh of tiles [k-4..k-1] fires when (k-1) completes a quad
# quad-synchronized epilogue: when tile k-1 completes quad q, the whole
# quad's vdot fires at once, reduces next step, exp the step after.
L_VDOT = 2                # quad vdot burst fires one step after its tanh
L_RED = 3                 # quad reduce one step after quad vdot
L_EXP = 4                 # quad exp two steps after quad vdot
L_CTX = 7                 # ctx pair (k-7, k-6) on even kc
L_END = 9


def _build_kernel(bl=BL, t_tiles=TT):
    nc = bacc.Bacc(
        "TRN2",
        target_bir_lowering=False,
        debug=False,
        num_devices=CORES,
    )

    # [b, tl, i, e]: x16[b, tl, i, :] = enc[b, i*128+tl, :] in bf16
    x16 = nc.declare_dram_parameter("x16", [bl, P, t_tiles, E], BF16, isOutput=False)
    # [b, el, i, j, tl]: xt8[b, el, i, j, tl] = enc[b, i*128+tl, j*128+el] in fp8
    xt8 = nc.declare_dram_parameter("xt8", [bl, P, t_tiles, ET, P], FP8, isOutput=False)
    # [el, j, d]: wet8[el, j, d] = 32 * We[d, j*128+el] in fp8
    wet8 = nc.declare_dram_parameter("wet8", [P, ET, D], FP8, isOutput=False)
    # [k, b, d]: 32 * z[b, d] in bf16, replicated over k partitions
    zrep16 = nc.declare_dram_parameter("zrep16", [P, bl, D], BF16, isOutput=False)
    v16 = nc.declare_dram_parameter("v16", [P, D], BF16, isOutput=False)
    onesc = nc.declare_dram_parameter("onesc", [P, 1], BF16, isOutput=False)
    out = nc.declare_dram_parameter("ctx_out", [bl, E], F32, isOutput=True)

    n_quads_total = bl * t_tiles // QUAD

    with tile.TileContext(nc) as tc, ExitStack() as ctx:
        const = ctx.enter_context(tc.tile_pool(name="const", bufs=1))
        xq_pool = ctx.enter_context(tc.tile_pool(name="xq", bufs=6))
        xtq_pool = ctx.enter_context(tc.tile_pool(name="xtq", bufs=6))
        epool = ctx.enter_context(tc.tile_pool(name="e", bufs=5))
        small = ctx.enter_context(tc.tile_pool(name="small", bufs=2))

        n_proj_bufs = 3 if K_PJ2 else 5
        ps_proj = ctx.enter_context(
            tc.tile_pool(name="ps_proj", bufs=n_proj_bufs, space="PSUM"))
        ps_ctx = ctx.enter_context(tc.tile_pool(name="ps_ctx", bufs=2, space="PSUM"))
        if not (K_PJ2 and K_CTXCG):
            ps_misc = ctx.enter_context(
                tc.tile_pool(name="ps_misc", bufs=1, space="PSUM"))

        # ---- resident constants.  Ordered so proj(0)'s inputs land first.
        xq_tiles = {}
        xtq_tiles = {}

        def fetch_quad(q, skip_xq=False):
            b, qi = divmod(q, t_tiles // QUAD)
            if not skip_xq:
                xq = xq_pool.tile([P, QUAD, E], BF16, tag="xq")
                nc.sync.dma_start(xq[:], x16[b, :, QUAD * qi : QUAD * (qi + 1), :])
                xq_tiles[q] = xq
            xtq = xtq_pool.tile([P, QUAD, ET, P], FP8, tag="xtq")
            nc.sync.dma_start(xtq[:], xt8[b, :, QUAD * qi : QUAD * (qi + 1), :, :])
            xtq_tiles[q] = xtq

        fetch_quad(0, skip_xq=True)
        wet_t = []
        z_t = [const.tile([P, D], BF16, name=f"z{b}") for b in range(bl)]
        for pr in range(ET // 2):
            wt = const.tile([P, 2, D], FP8, name=f"wet{pr}")
            nc.sync.dma_start(wt[:], wet8[:, 2 * pr : 2 * pr + 2, :])
            wet_t.append(wt)
            if pr == 1:
                nc.sync.dma_start(z_t[0][:], zrep16[:, 0])
        v_sb = const.tile([P, D], BF16)
        nc.sync.dma_start(v_sb[:], v16[:])
        xq0 = xq_pool.tile([P, QUAD, E], BF16, tag="xq")
        nc.sync.dma_start(xq0[:], x16[0, :, 0:QUAD, :])
        xq_tiles[0] = xq0
        onesc_sb = const.tile([P, 1], BF16)
        nc.sync.dma_start(onesc_sb[:], onesc[:])
        # scratch sinks (contents never read)
        junk = const.tile([P, D], BF16, name="junk")
        prod = const.tile([P, 4, D - ASPLIT], BF16, name="prod")

        # ---- per-batch state ------------------------------------------------
        total = bl * t_tiles
        state = {}
        equads = {}   # quad index -> [128, QUAD, D] e-buffer

        def get_state(b):
            if b not in state:
                state[b] = dict(
                    s0=small.tile([P, t_tiles], F32, tag="s0", name=f"s0_{b}"),
                    s1=small.tile([P, t_tiles], F32, tag="s1", name=f"s1_{b}"),
                    s=small.tile([P, t_tiles], F32, tag="s", name=f"s_{b}"),
                    p_all=small.tile([P, t_tiles], BF16, tag="p", name=f"p_all_{b}"),
                    # one PSUM bank: ctx halves at partitions 0 / 32, l at 64
                    ctxb=(ps_ctx.tile([P, 512], F32, tag="ps_ctx", name=f"ctxb_{b}")
                          if K_CTXCG else None),
                    ctx0=(None if K_CTXCG else
                          ps_ctx.tile([1, 512], F32, tag="ps_ctx", name=f"ctx0_{b}")),
                    ctx1=(None if K_CTXCG else
                          ps_ctx.tile([1, 512], F32, tag="ps_ctx", name=f"ctx1_{b}")),
                    proj_ps=[None] * t_tiles,
                )
            return state[b]

        def emit_proj(b, i):
            # proj[t, d] = sum_e x[t, e] * 32*WeT[e, d], fp8 DoubleRow,
            # both 512-halves into one [128,1024] 2-bank PSUM tile
            st = get_state(b)
            k = b * t_tiles + i
            q, qi = divmod(k, QUAD)
            xtq = xtq_tiles[q]
            if K_PJ2:
                pj = ps_proj.tile([P, D], F32, tag="ps_proj", name=f"pj_{b}_{i}")
                pjs = [pj[:, 0:512], pj[:, 512:1024]]
                st["proj_ps"][i] = [pj]
            else:
                pj0 = ps_proj.tile([P, 512], F32, tag="ps_proj", name=f"pj0_{b}_{i}")
                pj1 = ps_proj.tile([P, 512], F32, tag="ps_proj", name=f"pj1_{b}_{i}")
                pjs = [pj0[:], pj1[:]]
                st["proj_ps"][i] = [pj0, pj1]
            for pr in range(ET // 2):
                lhs = xtq[:, qi, 2 * pr : 2 * pr + 2, :]
                for h in range(2):
                    nc.tensor.matmul(
                        pjs[h], lhs, wet_t[pr][:, :, 512 * h : 512 * (h + 1)],
                        start=(pr == 0), stop=(pr == ET // 2 - 1), perf_mode=DR,
                    )

        def emit_zadd(k):
            # e = (proj + 32z) bf16 into this quad's slice; one [128,1024]
            # DVE op spanning both PSUM banks
            b, i = divmod(k, t_tiles)
            st = get_state(b)
            q, qi = divmod(k, QUAD)
            if q not in equads:
                equads[q] = epool.tile([P, QUAD, D], BF16, tag="e", name=f"equad_{q}")
            pjl = st["proj_ps"][i]
            st["proj_ps"][i] = None
            if K_PJ2:
                nc.vector.tensor_add(equads[q][:, qi, :], pjl[0][:], z_t[b][:])
            else:
                for h in range(2):
                    sl = slice(512 * h, 512 * (h + 1))
                    nc.vector.tensor_add(
                        equads[q][:, qi, sl], pjl[h][:], z_t[b][:, sl]
                    )

        def emit_tanh_tile(k):
            # drain tail: per-tile tanh right after its z-add spreads the
            # last quad's work before proj ends (ACT is idle there)
            q, qi = divmod(k, QUAD)
            nc.scalar.activation(
                equads[q][:, qi, :], equads[q][:, qi, :],
                mybir.ActivationFunctionType.Tanh, scale=1.0 / WSCALE,
            )

        def emit_tanh_quad(q):
            # tanh over 4 tiles at once; ~0.98us/tile vs 1.36 single
            if K_TANHQ:
                nc.scalar.activation(
                    equads[q][:], equads[q][:], mybir.ActivationFunctionType.Tanh,
                    scale=1.0 / WSCALE,
                )
            else:
                for qi in range(QUAD):
                    nc.scalar.activation(
                        equads[q][:, qi, :], equads[q][:, qi, :],
                        mybir.ActivationFunctionType.Tanh, scale=1.0 / WSCALE,
                    )

        def emit_vdot(k):
            # cols [0:ASPLIT): DVE STT fused mult+accum -> s0
            # cols [ASPLIT:D): Pool mult -> prod (ACT reduces next step)
            # Drain tail (last 8 tiles): whole v-dot on DVE so the ACT/Pool
            # queues empty sooner -- the drain critical path is ACT.
            b, i = divmod(k, t_tiles)
            st = get_state(b)
            q, qi = divmod(k, QUAD)
            e_sb = equads[q]
            asp = D if k >= total - 8 else ASPLIT
            nc.vector.scalar_tensor_tensor(
                out=junk[:, 0:asp],
                in0=e_sb[:, qi, 0:asp],
                scalar=1.0,
                in1=v_sb[:, 0:asp],
                op0=mybir.AluOpType.mult,
                op1=mybir.AluOpType.mult,
                accum_out=st["s0"][:, i : i + 1],
            )
            if asp < D:
                nc.gpsimd.tensor_mul(
                    prod[:, k % 4, :], e_sb[:, qi, asp:D], v_sb[:, asp:D]
                )

        def emit_reduce(k):
            # ACT Copy+accum over Pool's product half -> s1
            if k >= total - 8:
                return
            b, i = divmod(k, t_tiles)
            st = get_state(b)
            nc.scalar.activation(
                prod[:, k % 4, :],
                prod[:, k % 4, :],
                mybir.ActivationFunctionType.Copy,
                accum_out=st["s1"][:, i : i + 1],
            )

        def emit_exp4(b, i0):
            # s = s0+s1 (DVE, [P,4]) then p = exp(s) (one ACT instr)
            st = get_state(b)
            sl = slice(i0, i0 + 4)
            if b * t_tiles + i0 >= total - 8:
                src_s = st["s0"]          # tail tiles: s0 holds the full dot
            else:
                nc.gpsimd.tensor_add(
                    st["s"][:, sl], st["s0"][:, sl], st["s1"][:, sl]
                )
                src_s = st["s"]
            nc.scalar.activation(
                st["p_all"][:, sl], src_s[:, sl],
                mybir.ActivationFunctionType.Exp,
            )

        def emit_ctx(b, i):
            # ctx_unnorm += p^T @ X; halves go to partitions 0 / 32 of one
            # PSUM bank = different PE column groups -> concurrent
            st = get_state(b)
            k = b * t_tiles + i
            q, qi = divmod(k, QUAD)
            xq = xq_tiles[q]
            p_col = st["p_all"][:, i : i + 1]
            for h in range(2):
                if K_CTXCG:
                    dst = st["ctxb"][32 * h : 32 * h + 1, :]
                else:
                    dst = st["ctx0" if h == 0 else "ctx1"][:]
                nc.tensor.matmul(
                    dst, p_col,
                    xq[:, qi, h * 512 : (h + 1) * 512],
                    start=(i == 0), stop=(i == t_tiles - 1),
                )

        def emit_batch_l(b):
            # stage 1: l = sum_t exp(s_t) via DVE reduce + 1-col matmul into
            # partition 64 of the ctx bank.  Emitted 3 steps before the
            # normalize so the cross-engine chain spreads across the queues
            # (the l matmul lands inside the same step's bf16 ctx group).
            st = get_state(b)
            l_part = small.tile([P, 1], BF16, tag="lp")
            with nc.allow_low_precision(reason="l partials; err ~0.2%/sqrt(128)"):
                nc.vector.tensor_reduce(
                    l_part[:], st["p_all"][:],
                    axis=mybir.AxisListType.X, op=mybir.AluOpType.add,
                )
            l_ps = st["ctxb"][64:65, 0:1]
            nc.tensor.matmul(l_ps, l_part[:], onesc_sb[:])

        def emit_batch_end(b):
            # stage 2: ctx = ctx_unnorm / l
            st = state.pop(b)
            l_ps = st["ctxb"][64:65, 0:1]
            c0, c1 = st["ctxb"][0:1, :], st["ctxb"][32:33, :]
            linv = small.tile([1, 1], F32, tag="linv")
            nc.vector.reciprocal(linv[:], l_ps)
            ctx_row = small.tile([1, E], F32, tag="ctxrow")
            nc.scalar.activation(
                ctx_row[:, 0:512], c0,
                mybir.ActivationFunctionType.Copy, scale=linv[:],
            )
            nc.scalar.activation(
                ctx_row[:, 512:E], c1,
                mybir.ActivationFunctionType.Copy, scale=linv[:],
            )
            nc.sync.dma_start(out[b : b + 1, :], ctx_row[:])

        # ---- main software pipeline over all (batch, t-tile) ----------------
        for k in range(total + L_END + 2):
            if k < total:
                emit_proj(*divmod(k, t_tiles))
            if k == 0:
                for q in range(1, PREFETCH_QUADS):
                    fetch_quad(q)
            if k % QUAD == 0:
                qf = k // QUAD + PREFETCH_QUADS
                if qf < n_quads_total:
                    fetch_quad(qf)
            if k % t_tiles == 8 and k // t_tiles + 1 < bl:
                b_next = k // t_tiles + 1
                nc.sync.dma_start(z_t[b_next][:], zrep16[:, b_next])
            kz = k - L_ZADD
            if 0 <= kz < total:
                emit_zadd(kz)
                if kz >= total - QUAD:
                    emit_tanh_tile(kz)
                elif kz % QUAD == QUAD - 1:
                    emit_tanh_quad(kz // QUAD)
            kv = k - L_VDOT
            if 0 <= kv < total:
                if kv >= total - QUAD:
                    if kv == total - QUAD:
                        emit_vdot(kv - 2)
                        emit_vdot(kv - 1)
                    emit_vdot(kv)
                elif kv % QUAD == QUAD - 1:
                    emit_vdot(kv - 3)
                    emit_vdot(kv - 2)
                elif kv % QUAD == 0 and kv > 0:
                    emit_vdot(kv - 2)
                    emit_vdot(kv - 1)
            kr = k - L_RED
            if 0 <= kr < total and kr % QUAD == QUAD - 1:
                for kk in range(kr - 3, kr + 1):
                    emit_reduce(kk)
            ke = k - L_EXP
            if 0 <= ke < total and ke % 4 == 3:
                b_e, i_e = divmod(ke, t_tiles)
                emit_exp4(b_e, (i_e // 4) * 4)
            kc = k - L_CTX
            if kc >= 0 and kc % 2 == 0:
                for kk in (kc, kc + 1):
                    if 0 <= kk < total:
                        emit_ctx(*divmod(kk, t_tiles))
            kb1 = k - L_END + 3
            if 0 <= kb1 < total and kb1 % t_tiles == t_tiles - 1:
                emit_batch_l(kb1 // t_tiles)
            kb = k - L_END
            if 0 <= kb < total and kb % t_tiles == t_tiles - 1:
                emit_batch_end(kb // t_tiles)
            # free the consumed e-quad once its last tile's reduce is done
            kq = k - L_RED - 1
            if kq >= 0 and kq % QUAD == QUAD - 1 and (kq // QUAD) in equads:
                equads.pop(kq // QUAD)

    nc.compile()
    return nc


def _prep_inputs(enc_out, dec_state, W_weight, W_bias, v_weight, bl=BL):
    """Host-side layout/dtype prep + per-core slicing."""
    enc_out = np.ascontiguousarray(enc_out, dtype=np.float32)
    dec_state = np.ascontiguousarray(dec_state, dtype=np.float32)
    W = np.asarray(W_weight, dtype=np.float32)

    # x16: [B, tl, i, e] bf16
    x16_h = np.ascontiguousarray(
        enc_out.reshape(B, TT, P, E).transpose(0, 2, 1, 3).astype(NP_BF16)
    )
    # xt8: [B, el, i, j, tl] fp8
    enc8 = enc_out.astype(NP_FP8)
    xt8_h = np.ascontiguousarray(
        enc8.reshape(B, TT, P, ET, P).transpose(0, 4, 1, 3, 2)
    )
    # wet8: [el, j, d], scaled by WSCALE to avoid e4m3 subnormals
    wet8_h = np.ascontiguousarray(
        (WSCALE * W[:, :E].T).astype(NP_FP8).reshape(ET, P, D).transpose(1, 0, 2)
    )
    # z = Wd @ dec + bias, scaled by WSCALE, bf16, replicated over k
    z_all = dec_state @ W[:, E:].T + np.asarray(W_bias, dtype=np.float32)  # [B, D]
    z16 = (WSCALE * z_all).astype(NP_BF16)
    v16_h = np.ascontiguousarray(
        np.broadcast_to(np.asarray(v_weight).astype(NP_BF16).reshape(1, D), (P, D))
    )
    onesc_h = np.ones((P, 1), dtype=NP_BF16)

    in_maps = []
    for c in range(CORES):
        sl = slice(c * bl, (c + 1) * bl)
        zrep_h = np.ascontiguousarray(np.broadcast_to(z16[None, sl], (P, bl, D)))
        in_maps.append(
            {
                "x16": x16_h[sl],
                "xt8": xt8_h[sl],
                "wet8": wet8_h,
                "zrep16": zrep_h,
                "v16": v16_h,
                "onesc": onesc_h,
            }
        )
    return in_maps


_NC_CACHE = {}


def _get_nc():
    if "nc" not in _NC_CACHE:
        _NC_CACHE["nc"] = _build_kernel()
    return _NC_CACHE["nc"]


def _run(inputs, trace=False, tmpdir=None):
    nc = _get_nc()
    in_maps = _prep_inputs(
        inputs["enc_out"],
        inputs["dec_state"],
        inputs["W_weight"],
        inputs["W_bias"],
        inputs["v_weight"],
    )
    res = run_bass_kernel_spmd(
        nc, in_maps, list(range(CORES)), trace=trace, tmpdir=tmpdir
    )
    out = np.concatenate(
        [np.asarray(res.results[c]["ctx_out"]) for c in range(CORES)], axis=0
    )
    return out.astype(np.float32, copy=False), res


def kernel(**inputs):
    out, _ = _run(inputs, trace=False)
    return out


if __name__ == "__main__":
    pass
